# revision 28
# baseline (speedup 1.0000x reference)
"""Bass/Trainium2 kernel for nn_DreamAttention (dense transformer attention,
dead-softmax variant).

Math (per reference): q/k/v linear projections + RoPE, scores = q @ k^T /
sqrt(HD) (softmax computed but DISCARDED in the source), out = (scores @ v)
@ Wo^T.

Because no softmax is applied, attention is linear:
    (q @ k^T) @ v == q @ (k^T @ v)
so we compute the tiny per-head Gram matrix KV = k^T v  [HD, HD] instead of
the S x S score matrix (16x fewer FLOPs, no S x S materialization).

The q-side RoPE is folded into the attention matmul (RoPE is linear):
    attn_h = KV_h^T (cos*q_h) + KVp_h^T (sin* * q_h)
where KVp is KV with its partition halves swapped and sin* carries the
rotate-half signs. This lets the q projection emit feature-major tiles
directly (weight-stationary matmul), avoiding a transpose stage.

Sharding: data-parallel over tokens. 8 cores x 512 tokens (cores 0-3 hold
batch 0, cores 4-7 batch 1). Each core computes q/k/v for its tokens
(weights replicated), partial per-head KV over its tokens, ONE AllReduce of
the packed KV block within each 4-core batch group (bf16 payload, hidden
under the q projection), then attn and the output projection for its
tokens. The scale 1/sqrt(HD) is folded into k's RoPE tables on the host.

All matmul operands are bf16 (fp32 PSUM accumulation): same PE rate as
fp32r but half the HBM traffic, half the collective bytes, and full rate
at any moving-dim width (the per-head gram runs N=128). Projections use
4-PSUM-bank rounds so consecutive rounds double-buffer through the 8
banks. DMA triggers are spread across the SP/ACT/POOL sequencers.
"""

import math
from contextlib import ExitStack

import ml_dtypes
import numpy as np

import concourse.mybir as mybir
import concourse.tile as tile
from concourse import bacc
from concourse import bass_utils

P = 128
HD = 128
F32 = mybir.dt.float32
BF16 = mybir.dt.bfloat16


def ts(i, size):
    return slice(i * size, (i + 1) * size)


def emit_attn(tc, ctx, io, t_core, d_model, replica_groups):
    """Emit the per-core attention kernel.

    io: DRAM APs (bf16 unless noted): xT [d_model, t_core];
    wqT/wkT/wvT/woT [d_model, d_model]; bkb/bvb [128, d_model] (broadcast
    biases); bqd [128, d_model/128] fp32 (bq feature-major per-tile);
    cosk/sinkf [t_core, HD] (token-major k tables, sign-folded +
    1/sqrt(HD) prescaled); cosqD/sinqD [128, t_core] (feature-major q
    tables, sinqD sign-folded); y [t_core, d_model] fp32.
    """
    nc = tc.nc
    T_TILES = t_core // P       # 4 token tiles
    DIN = d_model // P          # 16 feature tiles (contraction)
    NH = d_model // HD          # 16 heads
    NCH = d_model // 512        # 4 output chunks of 512
    h2 = HD // 2

    sb = ctx.enter_context(tc.tile_pool(name="sb", bufs=1))
    ps = ctx.enter_context(tc.tile_pool(name="ps", bufs=8, space="PSUM"))
    dram = ctx.enter_context(tc.tile_pool(name="dram", bufs=2, space="DRAM"))

    def sbt(name, w, dtype=BF16, tag="gen", bufs=8):
        return sb.tile([P, w], dtype, name=name, tag=tag, bufs=bufs)

    def psum(name, width, dtype=F32, tag="ps", bufs=6):
        return ps.tile([P, width], dtype, name=name, tag=tag, bufs=bufs)

    # ---- resident x^T tiles [din, t] (operand for all four projections).
    # DMAs are emitted inside the first K-projection round, interleaved
    # with that round's weight DMAs, so the PE isn't starved at the head
    # (the DMA queues drain in submission order).
    xt_tiles = [None] * DIN
    ck_tiles, sk_tiles = [], []
    cosq = sinq = None

    def _load_tables():
        nonlocal cosq, sinq
        for t in range(T_TILES):
            ct = sbt(f"cosk{t}", HD, tag="tblk", bufs=2 * T_TILES)
            st = sbt(f"sink{t}", HD, tag="tblk", bufs=2 * T_TILES)
            nc.gpsimd.dma_start(ct[:], io["cosk"][ts(t, P), :])
            nc.gpsimd.dma_start(st[:], io["sinkf"][ts(t, P), :])
            ck_tiles.append(ct)
            sk_tiles.append(st)
        cosq = sbt("cosq", t_core, tag="tblq", bufs=2)
        sinq = sbt("sinq", t_core, tag="tblq", bufs=2)
        nc.gpsimd.dma_start(cosq[:], io["cosqD"][:])
        nc.gpsimd.dma_start(sinq[:], io["sinqD"][:])

    def project_tmajor(wT_ap, bias_ap, out_tiles, dma_eng, rope=False,
                       load_x=False, per_round=None):
        """out[t, dout] = x @ W^T + b into token-major tiles [128, d_model].

        One 512-wide chunk per round -> 4 PSUM banks, so consecutive
        rounds double-buffer through the 8 banks. If rope, applies the
        k-side RoPE to each finished [128, 512] slab (4 heads) in place.
        per_round(ch) is emitted after each round (gram pipelining).
        """
        for ch in range(NCH):
            psums = [psum(f"pp{t}", 512) for t in range(T_TILES)]
            for din in range(DIN):
                if load_x and ch == 0:
                    xt = sbt(f"xt{din}", t_core, tag="xt", bufs=DIN)
                    eng = nc.gpsimd if din % 2 else nc.scalar
                    if din == 0:
                        # first LDW only needs cols 0:128 — land it early
                        eng.dma_start(xt[:, 0:P], io["xT"][0:P, 0:P])
                        eng.dma_start(xt[:, P:], io["xT"][0:P, P:])
                    else:
                        eng.dma_start(xt[:], io["xT"][ts(din, P), :])
                    xt_tiles[din] = xt
                wt = sbt(f"w{din}", 512, tag="w", bufs=32)
                dma_eng.dma_start(wt[:], wT_ap[ts(din, P), ts(ch, 512)])
                for t in range(T_TILES):
                    nc.tensor.matmul(
                        psums[t][:],
                        xt_tiles[din][:, ts(t, P)],
                        wt[:],
                        start=(din == 0),
                        stop=(din == DIN - 1),
                    )
            if load_x and ch == 0:
                _load_tables()
            bt = sbt(f"bias{ch}", 512, tag="bias", bufs=4)
            dma_eng.dma_start(bt[:], bias_ap[:, ts(ch, 512)])
            for t in range(T_TILES):
                nc.vector.tensor_add(
                    out_tiles[t][:, ts(ch, 512)], psums[t][:], bt[:])
            if rope:
                for t in range(T_TILES):
                    _rope_slab(out_tiles[t], ch, ck_tiles[t], sk_tiles[t])
            if per_round is not None:
                per_round(ch)

    def _rope_slab(ktile, ch, ct, st):
        """In-place RoPE on a [128, 512] slab (4 heads) of a token-major
        tile: out = x*cos + rot_half(x)*sin, with sinf sign-folded so
        rot_half(x)*sin == gather(x, +-64) * sinf elementwise."""
        HPC = 512 // HD  # heads per chunk
        tmp = sbt(f"ropetmp{ch}", 512, tag="rtmp", bufs=2)
        x3 = ktile[:, ts(ch, 512)].rearrange("p (h d) -> p h d", d=HD)
        t3 = tmp[:].rearrange("p (h d) -> p h d", d=HD)

        def bc(ap2d):
            return ap2d.unsqueeze(1).broadcast_to([P, HPC, ap2d.shape[-1]])

        nc.vector.tensor_mul(t3[:, :, 0:h2], x3[:, :, h2:HD], bc(st[:, 0:h2]))
        nc.vector.tensor_mul(t3[:, :, h2:HD], x3[:, :, 0:h2], bc(st[:, h2:HD]))
        nc.vector.tensor_mul(x3, x3, bc(ct[:]))
        nc.vector.tensor_add(ktile[:, ts(ch, 512)], ktile[:, ts(ch, 512)],
                             tmp[:])

    # ---- K projection + RoPE(k) ----
    k_tiles = [sbt(f"k{t}", d_model, tag="kv", bufs=11) for t in range(T_TILES)]
    project_tmajor(io["wkT"], io["bkb"], k_tiles, nc.sync, rope=True,
                   load_x=True)

    # ---- V projection, with the per-head Gram matrices KV[h] = k_h^T v_h
    # (partial over this core's tokens) pipelined per finished 512-chunk
    # (4 heads), each slice DMA'd to DRAM as it completes so the
    # AllReduce launches right after the last V round ----
    v_tiles = [sbt(f"v{t}", d_model, tag="kv", bufs=11) for t in range(T_TILES)]
    kv_sb = sbt("kvsb", d_model, tag="kv", bufs=11)
    # separate contiguous DRAM tensors per collective half
    kv_in = [dram.tile([P, d_model // 2], BF16, name=f"kv_in{i}")
             for i in range(2)]
    kv_out = [dram.tile([P, d_model // 2], BF16, name=f"kv_out{i}")
              for i in range(2)]

    kv_red = sbt("kvred", d_model, tag="kv", bufs=11)
    kv_perm = sbt("kvperm", d_model, tag="kv", bufs=11)

    def _kv_fetch(g):
        """Post-collective copies for one 4-head group, on three different
        queues so they overlap. kv_perm is the partition-half-swapped copy
        for the folded q-side RoPE."""
        sl = ts(g, 512)
        src = kv_out[g // 2][:, ts(g % 2, 512)]
        nc.gpsimd.dma_start(kv_red[:, sl], src)
        nc.scalar.dma_start(kv_perm[0:h2, sl], src[h2:HD, :])
        nc.sync.dma_start(kv_perm[h2:HD, sl], src[0:h2, :])

    # The AllReduce is split in two 8-head halves, each launched as soon
    # as its gram inputs are complete (half 1 after V rounds 0-1, half 2
    # after the last V round). Both halves complete while the q
    # projection is still running, so the collective is fully hidden.
    def gram_group(ch):
        kvp = psum(f"kvp{ch}", 512)
        for j in range(4):
            h = 4 * ch + j
            for t in range(T_TILES):
                nc.tensor.matmul(
                    kvp[:, ts(j, HD)],
                    k_tiles[t][:, ts(h, HD)],
                    v_tiles[t][:, ts(h, HD)],
                    start=(t == 0),
                    stop=(t == T_TILES - 1),
                )
        nc.vector.tensor_copy(kv_sb[:, ts(ch, 512)], kvp[:])
        nc.gpsimd.dma_start(kv_in[ch // 2][:, ts(ch % 2, 512)],
                            kv_sb[:, ts(ch, 512)])
        if ch % 2 == 1:
            nc.gpsimd.collective_compute(
                "AllReduce",
                mybir.AluOpType.add,
                replica_groups=replica_groups,
                ins=[kv_in[ch // 2].opt()],
                outs=[kv_out[ch // 2].opt()],
            )
            _kv_fetch(ch - 1)
            _kv_fetch(ch)

    project_tmajor(io["wvT"], io["bvb"], v_tiles, nc.scalar,
                   per_round=gram_group)

    # ---- Q projection, feature-major: qD[dout, t] = W q-row blocks ----
    bqd_sb = sbt("bqd", DIN, dtype=F32, tag="bqd", bufs=1)
    nc.sync.dma_start(bqd_sb[:], io["bqd"][:])
    qcs = [None] * DIN
    for g in range(DIN // 4):
        psums = [psum(f"qp{j}", t_core) for j in range(4)]
        for din in range(DIN):
            wt = sbt(f"wq{din}", 512, tag="w", bufs=32)
            nc.sync.dma_start(wt[:], io["wqT"][ts(din, P), ts(g, 512)])
            for j in range(4):
                nc.tensor.matmul(
                    psums[j][:],
                    wt[:, ts(j, P)],
                    xt_tiles[din][:],
                    start=(din == 0),
                    stop=(din == DIN - 1),
                )
        for j in range(4):
            dout = g * 4 + j
            qd = sbt(f"qd{dout}", t_core, tag="qd", bufs=8)
            nc.vector.tensor_scalar_add(qd[:], psums[j][:],
                                        bqd_sb[:, dout:dout + 1])
            # RoPE multiplies hoisted here so they overlap the collective
            # wait; the attention phase is then pure PE.
            qc = sbt(f"qc{dout}", t_core, tag="qcs", bufs=2 * DIN)
            nc.vector.tensor_mul(qc[:], qd[:], cosq[:])
            qs = sbt(f"qs{dout}", t_core, tag="qcs", bufs=2 * DIN)
            nc.vector.tensor_mul(qs[:], qd[:], sinq[:])
            qcs[dout] = (qc, qs)

    # ---- attention + output projection, interleaved with the collective:
    # attn_h[d2, t] = KV_h^T (cos*q_h) + KVp_h^T (sin* q_h). Heads 0-7
    # (collective half 1) are emitted mid-q-projection; while half 2 is
    # still in flight the PE runs the dmid 0-7 half of the first output
    # chunk's contraction, then finishes when heads 8-15 land. ----
    attn_tiles = [None] * NH

    def attn_heads(h_lo, h_hi):
        for h in range(h_lo, h_hi):
            qc, qs = qcs[h]
            ap = psum(f"ap{h}", t_core)
            nc.tensor.matmul(ap[:], kv_red[:, ts(h, HD)], qc[:],
                             start=True, stop=False)
            nc.tensor.matmul(ap[:], kv_perm[:, ts(h, HD)], qs[:],
                             start=False, stop=True)
            asb = sbt(f"asb{h}", t_core, tag="attn", bufs=NH)
            nc.vector.tensor_copy(asb[:], ap[:])
            attn_tiles[h] = asb

    wo_pref = []
    for dmid in range(DIN):
        wt = sbt(f"wo{dmid}", 512, tag="w", bufs=32)
        nc.scalar.dma_start(wt[:], io["woT"][ts(dmid, P), 0:512])
        wo_pref.append(wt)

    attn_heads(0, NH // 2)

    # chunk-0 / token-tiles 0-1 partial contraction over heads 0-7: PE
    # work that does not depend on collective half 2, emitted while it is
    # still in flight. Held in a dedicated 2-bank psum tag so the attn
    # psum ring cannot cycle into it (that would deadlock the in-order
    # PE queue).
    out_engs = (nc.gpsimd, nc.sync, nc.scalar)
    held = [psum(f"op0_{t}", 512, tag="psO", bufs=2) for t in range(2)]
    for dmid in range(DIN // 2):
        for t in range(2):
            nc.tensor.matmul(
                held[t][:],
                attn_tiles[dmid][:, ts(t, P)],
                wo_pref[dmid][:],
                start=(dmid == 0),
                stop=False,
            )

    attn_heads(NH // 2, NH)

    for ch in range(NCH):
        for t in range(T_TILES):
            if ch == 0 and t < 2:
                pt = held[t]
                dmids = range(DIN // 2, DIN)
                first = None          # continues the held accumulation
            else:
                pt = psum(f"op{ch}_{t}", 512)
                dmids = range(DIN)
                first = 0
            for dmid in dmids:
                wt = wo_pref[dmid] if ch == 0 else wo_tiles[dmid]
                nc.tensor.matmul(
                    pt[:],
                    attn_tiles[dmid][:, ts(t, P)],
                    wt[:],
                    start=(dmid == first),
                    stop=(dmid == DIN - 1),
                )
            osb = sbt(f"osb{ch}_{t}", 512, dtype=F32, tag="osb", bufs=8)
            nc.vector.tensor_copy(osb[:], pt[:])
            out_engs[(ch * T_TILES + t) % 3].dma_start(
                io["y"][ts(t, P), ts(ch, 512)], osb[:])
        # weights for the next chunk
        if ch < NCH - 1:
            wo_tiles = []
            for dmid in range(DIN):
                wt = sbt(f"wo{dmid}", 512, tag="w", bufs=32)
                nc.scalar.dma_start(wt[:], io["woT"][ts(dmid, P),
                                                     ts(ch + 1, 512)])
                wo_tiles.append(wt)


def build_nc(t_core, d_model, num_devices, replica_groups, reps=1):
    nc = bacc.Bacc("TRN2", target_bir_lowering=False, debug=False,
                   num_devices=num_devices)
    io = {}
    io["xT"] = nc.dram_tensor("xT", [d_model, t_core], BF16,
                              kind="ExternalInput").ap()
    for nm in ("wqT", "wkT", "wvT", "woT"):
        io[nm] = nc.dram_tensor(nm, [d_model, d_model], BF16,
                                kind="ExternalInput").ap()
    for nm in ("bkb", "bvb"):
        io[nm] = nc.dram_tensor(nm, [P, d_model], BF16,
                                kind="ExternalInput").ap()
    io["bqd"] = nc.dram_tensor("bqd", [P, d_model // P], F32,
                               kind="ExternalInput").ap()
    for nm in ("cosk", "sinkf"):
        io[nm] = nc.dram_tensor(nm, [t_core, HD], BF16,
                                kind="ExternalInput").ap()
    for nm in ("cosqD", "sinqD"):
        io[nm] = nc.dram_tensor(nm, [P, t_core], BF16,
                                kind="ExternalInput").ap()
    io["y"] = nc.dram_tensor("y", [t_core, d_model], F32,
                             kind="ExternalOutput").ap()

    with tile.TileContext(nc) as tc:
        for _ in range(reps):
            with ExitStack() as ctx:
                emit_attn(tc, ctx, io, t_core, d_model, replica_groups)
    nc.compile()
    return nc


# ---------------- host side ----------------

B, S, D = 2, 2048, 2048
NH_FULL = 16
MAX_POS = 4096
ROPE_THETA = 10000.0
N_CORES = 8
T_CORE = B * S // N_CORES
REPLICA_GROUPS = [[0, 1, 2, 3], [4, 5, 6, 7]]

_cache = {}


def _bf(a):
    return np.ascontiguousarray(np.asarray(a, np.float32)).astype(
        ml_dtypes.bfloat16)


def _rope_tables():
    inv_freq = (np.float32(1.0) /
                np.power(np.float32(ROPE_THETA),
                         np.arange(0, HD, 2, dtype=np.float32) /
                         np.float32(HD))).astype(np.float32)
    t = np.arange(MAX_POS, dtype=np.float32)
    freqs = np.outer(t, inv_freq).astype(np.float32)
    emb = np.concatenate((freqs, freqs), axis=-1)
    return np.cos(emb).astype(np.float32), np.sin(emb).astype(np.float32)


def _get_nc():
    if "nc" not in _cache:
        _cache["nc"] = build_nc(T_CORE, D, N_CORES, REPLICA_GROUPS)
    return _cache["nc"]


def _host_inputs(hidden_states, position_ids, Wq, bq, Wk, bk, Wv, bv, Wo):
    x = np.asarray(hidden_states, dtype=np.float32).reshape(B * S, D)
    pos = np.asarray(position_ids).astype(np.int64).reshape(B * S)

    cos_t, sin_t = _rope_tables()
    cos = cos_t[pos]            # [B*S, HD]
    sin = sin_t[pos]
    # token-major k tables: sign-folded sin + 1/sqrt(HD) fold
    sinf = sin.copy()
    sinf[:, : HD // 2] *= np.float32(-1.0)
    scale = np.float32(1.0 / math.sqrt(HD))
    # feature-major q tables: sin* = +sin (i<64), -sin (i>=64)
    sinq = sin.copy()
    sinq[:, HD // 2:] *= np.float32(-1.0)

    wqT = _bf(np.asarray(Wq, np.float32).T)
    wkT = _bf(np.asarray(Wk, np.float32).T)
    wvT = _bf(np.asarray(Wv, np.float32).T)
    woT = _bf(np.asarray(Wo, np.float32).T)
    bkb = _bf(np.broadcast_to(np.asarray(bk, np.float32), (P, D)))
    bvb = _bf(np.broadcast_to(np.asarray(bv, np.float32), (P, D)))
    bqd = np.ascontiguousarray(
        np.asarray(bq, np.float32).reshape(D // P, P).T)

    in_maps = []
    for c in range(N_CORES):
        sl = slice(c * T_CORE, (c + 1) * T_CORE)
        in_maps.append({
            "xT": _bf(x[sl].T),
            "wqT": wqT, "wkT": wkT, "wvT": wvT, "woT": woT,
            "bkb": bkb, "bvb": bvb, "bqd": bqd,
            "cosk": _bf(cos[sl] * scale),
            "sinkf": _bf(sinf[sl] * scale),
            "cosqD": _bf(cos[sl].T),
            "sinqD": _bf(sinq[sl].T),
        })
    return in_maps


def kernel(hidden_states, position_ids, Wq, bq, Wk, bk, Wv, bv, Wo):
    in_maps = _host_inputs(hidden_states, position_ids,
                           Wq, bq, Wk, bk, Wv, bv, Wo)
    nc = _get_nc()
    last_err = None
    for attempt in range(3):
        try:
            res = bass_utils.run_bass_kernel_spmd(
                nc, in_maps, core_ids=list(range(N_CORES)))
            break
        except Exception as e:  # transient axon/device states clear on retry
            last_err = e
            import time
            time.sleep(15 * (attempt + 1))
    else:
        raise last_err
    out = np.concatenate([res.results[c]["y"] for c in range(N_CORES)], axis=0)
    return out.reshape(B, S, D)


# revision 31
# speedup vs baseline: 1.0021x; 1.0021x over previous
"""Bass/Trainium2 kernel for nn_DreamAttention (dense transformer attention,
dead-softmax variant).

Math (per reference): q/k/v linear projections + RoPE, scores = q @ k^T /
sqrt(HD) (softmax computed but DISCARDED in the source), out = (scores @ v)
@ Wo^T.

Because no softmax is applied, attention is linear:
    (q @ k^T) @ v == q @ (k^T @ v)
so we compute the tiny per-head Gram matrix KV = k^T v  [HD, HD] instead of
the S x S score matrix (16x fewer FLOPs, no S x S materialization).

The q-side RoPE is folded into the attention matmul (RoPE is linear):
    attn_h = KV_h^T (cos*q_h) + KVp_h^T (sin* * q_h)
where KVp is KV with its partition halves swapped and sin* carries the
rotate-half signs. This lets the q projection emit feature-major tiles
directly (weight-stationary matmul), avoiding a transpose stage.

Sharding: data-parallel over tokens. 8 cores x 512 tokens (cores 0-3 hold
batch 0, cores 4-7 batch 1). Each core computes q/k/v for its tokens
(weights replicated), partial per-head KV over its tokens, ONE AllReduce of
the packed KV block within each 4-core batch group (bf16 payload, hidden
under the q projection), then attn and the output projection for its
tokens. The scale 1/sqrt(HD) is folded into k's RoPE tables on the host.

All matmul operands are bf16 (fp32 PSUM accumulation): same PE rate as
fp32r but half the HBM traffic, half the collective bytes, and full rate
at any moving-dim width (the per-head gram runs N=128). Projections use
4-PSUM-bank rounds so consecutive rounds double-buffer through the 8
banks. DMA triggers are spread across the SP/ACT/POOL sequencers.
"""

import math
from contextlib import ExitStack

import ml_dtypes
import numpy as np

import concourse.mybir as mybir
import concourse.tile as tile
from concourse import bacc
from concourse import bass_utils

P = 128
HD = 128
F32 = mybir.dt.float32
BF16 = mybir.dt.bfloat16


def ts(i, size):
    return slice(i * size, (i + 1) * size)


def emit_attn(tc, ctx, io, t_core, d_model, replica_groups):
    """Emit the per-core attention kernel.

    io: DRAM APs (bf16 unless noted): xT [d_model, t_core];
    wqT/wkT/wvT/woT [d_model, d_model]; bkb/bvb [128, d_model] (broadcast
    biases); bqd [128, d_model/128] fp32 (bq feature-major per-tile);
    cosk/sinkf [t_core, HD] (token-major k tables, sign-folded +
    1/sqrt(HD) prescaled); cosqD/sinqD [128, t_core] (feature-major q
    tables, sinqD sign-folded); y [t_core, d_model] fp32.
    """
    nc = tc.nc
    T_TILES = t_core // P       # 4 token tiles
    DIN = d_model // P          # 16 feature tiles (contraction)
    NH = d_model // HD          # 16 heads
    NCH = d_model // 512        # 4 output chunks of 512
    h2 = HD // 2

    sb = ctx.enter_context(tc.tile_pool(name="sb", bufs=1))
    ps = ctx.enter_context(tc.tile_pool(name="ps", bufs=8, space="PSUM"))
    dram = ctx.enter_context(tc.tile_pool(name="dram", bufs=2, space="DRAM"))

    def sbt(name, w, dtype=BF16, tag="gen", bufs=8):
        return sb.tile([P, w], dtype, name=name, tag=tag, bufs=bufs)

    def psum(name, width, dtype=F32, tag="ps", bufs=6):
        return ps.tile([P, width], dtype, name=name, tag=tag, bufs=bufs)

    # ---- resident x^T tiles [din, t] (operand for all four projections).
    # DMAs are emitted inside the first K-projection round, interleaved
    # with that round's weight DMAs, so the PE isn't starved at the head
    # (the DMA queues drain in submission order).
    xt_tiles = [None] * DIN
    ck_tiles, sk_tiles = [], []
    cosq = sinq = None

    def _load_tables():
        nonlocal cosq, sinq
        for t in range(T_TILES):
            ct = sbt(f"cosk{t}", HD, tag="tblk", bufs=2 * T_TILES)
            st = sbt(f"sink{t}", HD, tag="tblk", bufs=2 * T_TILES)
            nc.gpsimd.dma_start(ct[:], io["cosk"][ts(t, P), :])
            nc.gpsimd.dma_start(st[:], io["sinkf"][ts(t, P), :])
            ck_tiles.append(ct)
            sk_tiles.append(st)
        cosq = sbt("cosq", t_core, tag="tblq", bufs=2)
        sinq = sbt("sinq", t_core, tag="tblq", bufs=2)
        nc.gpsimd.dma_start(cosq[:], io["cosqD"][:])
        nc.gpsimd.dma_start(sinq[:], io["sinqD"][:])

    def project_tmajor(wT_ap, bias_ap, out_tiles, dma_eng, rope=False,
                       load_x=False, per_round=None):
        """out[t, dout] = x @ W^T + b into token-major tiles [128, d_model].

        One 512-wide chunk per round -> 4 PSUM banks, so consecutive
        rounds double-buffer through the 8 banks. If rope, applies the
        k-side RoPE to each finished [128, 512] slab (4 heads) in place.
        per_round(ch) is emitted after each round (gram pipelining).
        """
        for ch in range(NCH):
            psums = [psum(f"pp{t}", 512) for t in range(T_TILES)]
            for din in range(DIN):
                if load_x and ch == 0:
                    xt = sbt(f"xt{din}", t_core, tag="xt", bufs=DIN)
                    eng = nc.gpsimd if din % 2 else nc.scalar
                    if din == 0:
                        # first LDW only needs cols 0:128 — land it early
                        eng.dma_start(xt[:, 0:P], io["xT"][0:P, 0:P])
                        eng.dma_start(xt[:, P:], io["xT"][0:P, P:])
                    else:
                        eng.dma_start(xt[:], io["xT"][ts(din, P), :])
                    xt_tiles[din] = xt
                wt = sbt(f"w{din}", 512, tag="w", bufs=32)
                dma_eng.dma_start(wt[:], wT_ap[ts(din, P), ts(ch, 512)])
                for t in range(T_TILES):
                    nc.tensor.matmul(
                        psums[t][:],
                        xt_tiles[din][:, ts(t, P)],
                        wt[:],
                        start=(din == 0),
                        stop=(din == DIN - 1),
                    )
            if load_x and ch == 0:
                _load_tables()
            bt = sbt(f"bias{ch}", 512, tag="bias", bufs=4)
            dma_eng.dma_start(bt[:], bias_ap[:, ts(ch, 512)])
            for t in range(T_TILES):
                nc.vector.tensor_add(
                    out_tiles[t][:, ts(ch, 512)], psums[t][:], bt[:])
            if rope:
                for t in range(T_TILES):
                    _rope_slab(out_tiles[t], ch, ck_tiles[t], sk_tiles[t])
            if per_round is not None:
                per_round(ch)

    def _rope_slab(ktile, ch, ct, st):
        """In-place RoPE on a [128, 512] slab (4 heads) of a token-major
        tile: out = x*cos + rot_half(x)*sin, with sinf sign-folded so
        rot_half(x)*sin == gather(x, +-64) * sinf elementwise."""
        HPC = 512 // HD  # heads per chunk
        tmp = sbt(f"ropetmp{ch}", 512, tag="rtmp", bufs=2)
        x3 = ktile[:, ts(ch, 512)].rearrange("p (h d) -> p h d", d=HD)
        t3 = tmp[:].rearrange("p (h d) -> p h d", d=HD)

        def bc(ap2d):
            return ap2d.unsqueeze(1).broadcast_to([P, HPC, ap2d.shape[-1]])

        nc.vector.tensor_mul(t3[:, :, 0:h2], x3[:, :, h2:HD], bc(st[:, 0:h2]))
        nc.vector.tensor_mul(t3[:, :, h2:HD], x3[:, :, 0:h2], bc(st[:, h2:HD]))
        nc.vector.tensor_mul(x3, x3, bc(ct[:]))
        nc.vector.tensor_add(ktile[:, ts(ch, 512)], ktile[:, ts(ch, 512)],
                             tmp[:])

    # ---- K projection + RoPE(k) ----
    k_tiles = [sbt(f"k{t}", d_model, tag="kv", bufs=11) for t in range(T_TILES)]
    project_tmajor(io["wkT"], io["bkb"], k_tiles, nc.sync, rope=True,
                   load_x=True)

    # ---- V projection, with the per-head Gram matrices KV[h] = k_h^T v_h
    # (partial over this core's tokens) pipelined per finished 512-chunk
    # (4 heads), each slice DMA'd to DRAM as it completes so the
    # AllReduce launches right after the last V round ----
    v_tiles = [sbt(f"v{t}", d_model, tag="kv", bufs=11) for t in range(T_TILES)]
    kv_sb = sbt("kvsb", d_model, tag="kv", bufs=11)
    # separate contiguous DRAM tensors per collective half
    kv_in = [dram.tile([P, d_model // 2], BF16, name=f"kv_in{i}")
             for i in range(2)]
    kv_out = [dram.tile([P, d_model // 2], BF16, name=f"kv_out{i}")
              for i in range(2)]

    kv_red = sbt("kvred", d_model, tag="kv", bufs=11)
    kv_perm = sbt("kvperm", d_model, tag="kv", bufs=11)

    def _kv_fetch(g):
        """Post-collective copies for one 4-head group, on three different
        queues so they overlap. kv_perm is the partition-half-swapped copy
        for the folded q-side RoPE."""
        sl = ts(g, 512)
        src = kv_out[g // 2][:, ts(g % 2, 512)]
        nc.gpsimd.dma_start(kv_red[:, sl], src)
        nc.scalar.dma_start(kv_perm[0:h2, sl], src[h2:HD, :])
        nc.sync.dma_start(kv_perm[h2:HD, sl], src[0:h2, :])

    # The AllReduce is split in two 8-head halves, each launched as soon
    # as its gram inputs are complete (half 1 after V rounds 0-1, half 2
    # after the last V round). Both halves complete while the q
    # projection is still running, so the collective is fully hidden.
    def gram_group(ch):
        # psO tag is idle until the output projection, so the gram psum
        # doesn't tighten the projection-round double-buffer ring
        kvp = psum(f"kvp{ch}", 512, tag="psO", bufs=2)
        for j in range(4):
            h = 4 * ch + j
            for t in range(T_TILES):
                nc.tensor.matmul(
                    kvp[:, ts(j, HD)],
                    k_tiles[t][:, ts(h, HD)],
                    v_tiles[t][:, ts(h, HD)],
                    start=(t == 0),
                    stop=(t == T_TILES - 1),
                )
        nc.vector.tensor_copy(kv_sb[:, ts(ch, 512)], kvp[:])
        nc.gpsimd.dma_start(kv_in[ch // 2][:, ts(ch % 2, 512)],
                            kv_sb[:, ts(ch, 512)])
        if ch % 2 == 1:
            nc.gpsimd.collective_compute(
                "AllReduce",
                mybir.AluOpType.add,
                replica_groups=replica_groups,
                ins=[kv_in[ch // 2].opt()],
                outs=[kv_out[ch // 2].opt()],
            )
            _kv_fetch(ch - 1)
            _kv_fetch(ch)

    project_tmajor(io["wvT"], io["bvb"], v_tiles, nc.scalar,
                   per_round=gram_group)

    # ---- Q projection, feature-major: qD[dout, t] = W q-row blocks ----
    bqd_sb = sbt("bqd", DIN, dtype=F32, tag="bqd", bufs=1)
    nc.sync.dma_start(bqd_sb[:], io["bqd"][:])
    qcs = [None] * DIN
    for g in range(DIN // 4):
        psums = [psum(f"qp{j}", t_core) for j in range(4)]
        for din in range(DIN):
            wt = sbt(f"wq{din}", 512, tag="w", bufs=32)
            nc.sync.dma_start(wt[:], io["wqT"][ts(din, P), ts(g, 512)])
            for j in range(4):
                nc.tensor.matmul(
                    psums[j][:],
                    wt[:, ts(j, P)],
                    xt_tiles[din][:],
                    start=(din == 0),
                    stop=(din == DIN - 1),
                )
        for j in range(4):
            dout = g * 4 + j
            qd = sbt(f"qd{dout}", t_core, tag="qd", bufs=8)
            nc.vector.tensor_scalar_add(qd[:], psums[j][:],
                                        bqd_sb[:, dout:dout + 1])
            # RoPE multiplies hoisted here so they overlap the collective
            # wait; the attention phase is then pure PE.
            qc = sbt(f"qc{dout}", t_core, tag="qcs", bufs=2 * DIN)
            nc.vector.tensor_mul(qc[:], qd[:], cosq[:])
            qs = sbt(f"qs{dout}", t_core, tag="qcs", bufs=2 * DIN)
            nc.vector.tensor_mul(qs[:], qd[:], sinq[:])
            qcs[dout] = (qc, qs)

    # ---- attention + output projection, interleaved with the collective:
    # attn_h[d2, t] = KV_h^T (cos*q_h) + KVp_h^T (sin* q_h). Heads 0-7
    # (collective half 1) are emitted mid-q-projection; while half 2 is
    # still in flight the PE runs the dmid 0-7 half of the first output
    # chunk's contraction, then finishes when heads 8-15 land. ----
    attn_tiles = [None] * NH

    def attn_heads(h_lo, h_hi):
        for h in range(h_lo, h_hi):
            qc, qs = qcs[h]
            ap = psum(f"ap{h}", t_core)
            nc.tensor.matmul(ap[:], kv_red[:, ts(h, HD)], qc[:],
                             start=True, stop=False)
            nc.tensor.matmul(ap[:], kv_perm[:, ts(h, HD)], qs[:],
                             start=False, stop=True)
            asb = sbt(f"asb{h}", t_core, tag="attn", bufs=NH)
            nc.vector.tensor_copy(asb[:], ap[:])
            attn_tiles[h] = asb

    wo_pref = []
    for dmid in range(DIN):
        wt = sbt(f"wo{dmid}", 512, tag="w", bufs=32)
        nc.scalar.dma_start(wt[:], io["woT"][ts(dmid, P), 0:512])
        wo_pref.append(wt)

    attn_heads(0, NH // 2)

    # chunk-0 / token-tiles 0-1 partial contraction over heads 0-7: PE
    # work that does not depend on collective half 2, emitted while it is
    # still in flight. Held in a dedicated 2-bank psum tag so the attn
    # psum ring cannot cycle into it (that would deadlock the in-order
    # PE queue).
    out_engs = (nc.gpsimd, nc.sync, nc.scalar)
    held = [psum(f"op0_{t}", 512, tag="psO", bufs=2) for t in range(2)]
    for dmid in range(DIN // 2):
        for t in range(2):
            nc.tensor.matmul(
                held[t][:],
                attn_tiles[dmid][:, ts(t, P)],
                wo_pref[dmid][:],
                start=(dmid == 0),
                stop=False,
            )

    attn_heads(NH // 2, NH)

    for ch in range(NCH):
        for t in range(T_TILES):
            if ch == 0 and t < 2:
                pt = held[t]
                dmids = range(DIN // 2, DIN)
                first = None          # continues the held accumulation
            else:
                pt = psum(f"op{ch}_{t}", 512)
                dmids = range(DIN)
                first = 0
            for dmid in dmids:
                wt = wo_pref[dmid] if ch == 0 else wo_tiles[dmid]
                nc.tensor.matmul(
                    pt[:],
                    attn_tiles[dmid][:, ts(t, P)],
                    wt[:],
                    start=(dmid == first),
                    stop=(dmid == DIN - 1),
                )
            osb = sbt(f"osb{ch}_{t}", 512, dtype=F32, tag="osb", bufs=8)
            nc.vector.tensor_copy(osb[:], pt[:])
            out_engs[(ch * T_TILES + t) % 3].dma_start(
                io["y"][ts(t, P), ts(ch, 512)], osb[:])
        # weights for the next chunk
        if ch < NCH - 1:
            wo_tiles = []
            for dmid in range(DIN):
                wt = sbt(f"wo{dmid}", 512, tag="w", bufs=32)
                nc.scalar.dma_start(wt[:], io["woT"][ts(dmid, P),
                                                     ts(ch + 1, 512)])
                wo_tiles.append(wt)


def build_nc(t_core, d_model, num_devices, replica_groups, reps=1):
    nc = bacc.Bacc("TRN2", target_bir_lowering=False, debug=False,
                   num_devices=num_devices)
    io = {}
    io["xT"] = nc.dram_tensor("xT", [d_model, t_core], BF16,
                              kind="ExternalInput").ap()
    for nm in ("wqT", "wkT", "wvT", "woT"):
        io[nm] = nc.dram_tensor(nm, [d_model, d_model], BF16,
                                kind="ExternalInput").ap()
    for nm in ("bkb", "bvb"):
        io[nm] = nc.dram_tensor(nm, [P, d_model], BF16,
                                kind="ExternalInput").ap()
    io["bqd"] = nc.dram_tensor("bqd", [P, d_model // P], F32,
                               kind="ExternalInput").ap()
    for nm in ("cosk", "sinkf"):
        io[nm] = nc.dram_tensor(nm, [t_core, HD], BF16,
                                kind="ExternalInput").ap()
    for nm in ("cosqD", "sinqD"):
        io[nm] = nc.dram_tensor(nm, [P, t_core], BF16,
                                kind="ExternalInput").ap()
    io["y"] = nc.dram_tensor("y", [t_core, d_model], F32,
                             kind="ExternalOutput").ap()

    with tile.TileContext(nc) as tc:
        for _ in range(reps):
            with ExitStack() as ctx:
                emit_attn(tc, ctx, io, t_core, d_model, replica_groups)
    nc.compile()
    return nc


# ---------------- host side ----------------

B, S, D = 2, 2048, 2048
NH_FULL = 16
MAX_POS = 4096
ROPE_THETA = 10000.0
N_CORES = 8
T_CORE = B * S // N_CORES
REPLICA_GROUPS = [[0, 1, 2, 3], [4, 5, 6, 7]]

_cache = {}


def _bf(a):
    return np.ascontiguousarray(np.asarray(a, np.float32)).astype(
        ml_dtypes.bfloat16)


def _rope_tables():
    inv_freq = (np.float32(1.0) /
                np.power(np.float32(ROPE_THETA),
                         np.arange(0, HD, 2, dtype=np.float32) /
                         np.float32(HD))).astype(np.float32)
    t = np.arange(MAX_POS, dtype=np.float32)
    freqs = np.outer(t, inv_freq).astype(np.float32)
    emb = np.concatenate((freqs, freqs), axis=-1)
    return np.cos(emb).astype(np.float32), np.sin(emb).astype(np.float32)


def _get_nc():
    if "nc" not in _cache:
        _cache["nc"] = build_nc(T_CORE, D, N_CORES, REPLICA_GROUPS)
    return _cache["nc"]


def _host_inputs(hidden_states, position_ids, Wq, bq, Wk, bk, Wv, bv, Wo):
    x = np.asarray(hidden_states, dtype=np.float32).reshape(B * S, D)
    pos = np.asarray(position_ids).astype(np.int64).reshape(B * S)

    cos_t, sin_t = _rope_tables()
    cos = cos_t[pos]            # [B*S, HD]
    sin = sin_t[pos]
    # token-major k tables: sign-folded sin + 1/sqrt(HD) fold
    sinf = sin.copy()
    sinf[:, : HD // 2] *= np.float32(-1.0)
    scale = np.float32(1.0 / math.sqrt(HD))
    # feature-major q tables: sin* = +sin (i<64), -sin (i>=64)
    sinq = sin.copy()
    sinq[:, HD // 2:] *= np.float32(-1.0)

    wqT = _bf(np.asarray(Wq, np.float32).T)
    wkT = _bf(np.asarray(Wk, np.float32).T)
    wvT = _bf(np.asarray(Wv, np.float32).T)
    woT = _bf(np.asarray(Wo, np.float32).T)
    bkb = _bf(np.broadcast_to(np.asarray(bk, np.float32), (P, D)))
    bvb = _bf(np.broadcast_to(np.asarray(bv, np.float32), (P, D)))
    bqd = np.ascontiguousarray(
        np.asarray(bq, np.float32).reshape(D // P, P).T)

    in_maps = []
    for c in range(N_CORES):
        sl = slice(c * T_CORE, (c + 1) * T_CORE)
        in_maps.append({
            "xT": _bf(x[sl].T),
            "wqT": wqT, "wkT": wkT, "wvT": wvT, "woT": woT,
            "bkb": bkb, "bvb": bvb, "bqd": bqd,
            "cosk": _bf(cos[sl] * scale),
            "sinkf": _bf(sinf[sl] * scale),
            "cosqD": _bf(cos[sl].T),
            "sinqD": _bf(sinq[sl].T),
        })
    return in_maps


def kernel(hidden_states, position_ids, Wq, bq, Wk, bk, Wv, bv, Wo):
    in_maps = _host_inputs(hidden_states, position_ids,
                           Wq, bq, Wk, bk, Wv, bv, Wo)
    nc = _get_nc()
    last_err = None
    for attempt in range(3):
        try:
            res = bass_utils.run_bass_kernel_spmd(
                nc, in_maps, core_ids=list(range(N_CORES)))
            break
        except Exception as e:  # transient axon/device states clear on retry
            last_err = e
            import time
            time.sleep(15 * (attempt + 1))
    else:
        raise last_err
    out = np.concatenate([res.results[c]["y"] for c in range(N_CORES)], axis=0)
    return out.reshape(B, S, D)
